# revision 11
# baseline (speedup 1.0000x reference)
"""VQ-EMA codebook update kernel for Trainium2, 8 NeuronCores.

Strategy (data-parallel over tokens, standard VQ-EMA sync):
  - Each core: N/8 = 4096 tokens; [K=8192, C=384] dictionary replicated.
  - Key insight: argmax_k <x/|x|, d_k/|d_k|> == argmax_k <x, d_k/|d_k|>, so
    features are never normalized; only the dictionary is (P0).
  - All sim-side data in fp16: ndT [C,K] fp16 (48KB/part), xT fp16 chunks.
  - P1 per 128-token tile: cast x->fp16 into resident xext (x | ones col),
    transpose x chunks on PE, sim = xT.T @ ndT in fp16 (1 cyc/row), PSUM->
    SBUF fp16 evacuations split ScalarE/DVE, rowmax via 2x tensor_tensor max
    tree + small 1x reduce, onehot = (sim >= rowmax) fp16 at 4x DVE rate,
    spilled to DRAM (64MB; xext stays resident).
  - P2: segment sums: per K-group, accumulate onehot.T @ xext over all token
    tiles in PSUM -> partial [K, C+1] (feature sums + counts).
  - ReduceScatter(add) per group across 8 cores -> each core owns [1024,C+1]
    shard rows; EMA update + where(used) blend on-core (P3).
Host: shards inputs, concatenates the 8 output shards.
"""

import sys

sys.path.insert(0, "/opt/trn_rl_repo")

import functools

import numpy as np

N = 32768
C = 384
K = 8192
NCORES = 8
NSH = N // NCORES  # 4096 tokens per core
KSH = K // NCORES  # 1024 dict rows per core
TT = NSH // 128  # 32 token tiles per core
KT = K // 128  # 64 K tiles
CB = C // 128  # 3 contraction chunks
SIMW = 512  # sim matmul free width (PSUM bank, fp32)
NSIMW = K // SIMW  # 16 chunks
XW = C + 1  # 385: x plus ones column
MOM = 0.99
N_SE_EVAC = 12  # sim chunks evacuated on ScalarE; rest on DVE


@functools.cache
def _build():
    import concourse.bacc as bacc
    import concourse.masks as masks
    import concourse.mybir as mybir
    import concourse.tile as tile

    f32 = mybir.dt.float32
    f16 = mybir.dt.float16

    nc = bacc.Bacc("TRN2", target_bir_lowering=False, debug=False, num_devices=NCORES)

    # fp16 inputs: the per-execution input staging costs ~17ns/byte, and the
    # kernel quantizes feat/dic to fp16 internally anyway. dsum/dsh at fp16
    # add ~2e-4 relative error, well within tolerance.
    feat = nc.dram_tensor("feat", [NSH, C], f16, kind="ExternalInput").ap()
    dic = nc.dram_tensor("dic", [K, C], f16, kind="ExternalInput").ap()
    dsum = nc.dram_tensor("dsum", [KSH, C], f16, kind="ExternalInput").ap()
    dnum = nc.dram_tensor("dnum", [KSH // 128, 128, 1], f32, kind="ExternalInput").ap()
    dsh = nc.dram_tensor("dsh", [KSH, C], f16, kind="ExternalInput").ap()
    out_shard = nc.dram_tensor("out_shard", [KSH, C], f32, kind="ExternalOutput").ap()

    with tile.TileContext(nc) as tc:
        with (
            tc.tile_pool(name="constp", bufs=1) as constp,
            tc.tile_pool(name="mainp", bufs=1) as mainp,
            tc.tile_pool(name="dramp", bufs=1, space="DRAM") as dramp,
        ):
            ident = constp.tile([128, 128], f16, name="ident")
            masks.make_identity(nc, ident[:])

            # Persistent SBUF tensors
            ndT = [
                mainp.tile([128, K], f16, name=f"ndT{c}", uniquify=False)
                for c in range(CB)
            ]
            # xext: fp16 features + ones column, resident across P1/P2
            xext = mainp.tile([128, TT, XW], f16, name="xext", uniquify=False)
            nc.vector.memset(xext[:, :, C:XW], 1.0)

            # DRAM scratch
            onehot_dram = dramp.tile([NSH, K], f16, name="onehot_dram")
            partial_dram = dramp.tile([K, XW], f32, name="partial_dram")
            ccout_dram = dramp.tile([KSH, XW], f32, name="ccout_dram")

            # ---------------- P0: dictionary normalize + transpose ----------------
            with (
                tc.tile_pool(name="p0sb", bufs=3) as p0sb,
                tc.tile_pool(name="p0sc", bufs=2) as p0sc,
                tc.tile_pool(name="p0ps", bufs=2, space="PSUM") as p0ps,
            ):
                for dt_i in range(KT):
                    d = p0sb.tile([128, C], f16, name="d", tag="d")
                    nc.sync.dma_start(d[:], dic[dt_i * 128 : (dt_i + 1) * 128, :])
                    sq = p0sc.tile([128, C], f32, name="sq", tag="sq")
                    ss = p0sc.tile([128, 1], f32, name="ss", tag="ss")
                    nc.scalar.activation(
                        sq[:], d[:], mybir.ActivationFunctionType.Square, accum_out=ss[:]
                    )
                    # r ~= 1/sqrt(ss): reciprocal+sqrt then one Newton step
                    rec = p0sc.tile([128, 1], f32, name="rec", tag="rec")
                    r0 = p0sc.tile([128, 1], f32, name="r0", tag="r0")
                    t = p0sc.tile([128, 1], f32, name="t", tag="t")
                    r = p0sc.tile([128, 1], f32, name="r", tag="r")
                    nc.vector.reciprocal(rec[:], ss[:])
                    nc.scalar.sqrt(r0[:], rec[:])
                    nc.vector.tensor_tensor(t[:], r0[:], r0[:], mybir.AluOpType.mult)
                    nc.vector.tensor_tensor(t[:], t[:], ss[:], mybir.AluOpType.mult)
                    nc.vector.tensor_scalar(
                        t[:], t[:], -0.5, 1.5, mybir.AluOpType.mult, mybir.AluOpType.add
                    )
                    nc.vector.tensor_tensor(r[:], r0[:], t[:], mybir.AluOpType.mult)
                    nd = p0sb.tile([128, C], f16, name="nd", tag="nd")
                    # normalize on DVE (2x single-src mode) to keep ScalarE light
                    nc.vector.tensor_scalar(
                        nd[:], d[:], r[:, 0:1], None, mybir.AluOpType.mult
                    )
                    for c in range(CB):
                        pst = p0ps.tile([128, 128], f16, name="pst", tag="pst")
                        nc.tensor.transpose(pst[:], nd[:, c * 128 : (c + 1) * 128], ident[:])
                        nc.vector.tensor_copy(
                            ndT[c][:, dt_i * 128 : (dt_i + 1) * 128], pst[:]
                        )

            # ---------------- P1: sim + one-hot per token tile ----------------
            with (
                tc.tile_pool(name="p1sb", bufs=4) as p1sb,
                tc.tile_pool(name="p1xt", bufs=3) as p1xt,
                tc.tile_pool(name="p1sim", bufs=2) as p1sim,
                tc.tile_pool(name="p1mx", bufs=2) as p1mx,
                tc.tile_pool(name="p1oh", bufs=2) as p1oh,
                tc.tile_pool(name="p1ps", bufs=4, space="PSUM") as p1ps,
                tc.tile_pool(name="p1pst", bufs=3, space="PSUM") as p1pst,
            ):
                for tt in range(TT):
                    # fp16 features DMA straight into the resident xext slot
                    nc.sync.dma_start(
                        xext[:, tt, 0:C], feat[tt * 128 : (tt + 1) * 128, :]
                    )

                    xT = []
                    for c in range(CB):
                        pst = p1pst.tile([128, 128], f16, name="pstx", tag="pstx")
                        nc.tensor.transpose(
                            pst[:], xext[:, tt, c * 128 : (c + 1) * 128], ident[:]
                        )
                        xc = p1xt.tile([128, 128], f16, name="xc", tag=f"xc{c}")
                        nc.scalar.copy(xc[:], pst[:])
                        xT.append(xc)

                    simbuf = p1sim.tile([128, K], f16, name="simbuf", tag="simbuf")
                    m1 = p1mx.tile([128, K // 2], f16, name="m1", tag="m1")
                    m2 = p1mx.tile([128, K // 4], f16, name="m2", tag="m2")
                    m3 = p1mx.tile([128, K // 8], f16, name="m3", tag="m3")
                    for kc in range(NSIMW):
                        ps = p1ps.tile([128, SIMW], f32, name="ps_sim", tag="ps_sim")
                        for c in range(CB):
                            nc.tensor.matmul(
                                ps[:],
                                xT[c][:],
                                ndT[c][:, kc * SIMW : (kc + 1) * SIMW],
                                start=(c == 0),
                                stop=(c == CB - 1),
                            )
                        dst = simbuf[:, kc * SIMW : (kc + 1) * SIMW]
                        if kc < N_SE_EVAC:
                            nc.scalar.copy(dst, ps[:])
                        else:
                            nc.vector.tensor_copy(dst, ps[:])
                        # pairwise max as soon as a chunk pair lands: small 2x
                        # ops pipeline with the matmuls and avoid DVE HOL stalls
                        if kc % 2 == 1:
                            j = kc // 2
                            nc.vector.tensor_tensor(
                                m1[:, j * SIMW : (j + 1) * SIMW],
                                simbuf[:, (kc - 1) * SIMW : kc * SIMW],
                                dst,
                                mybir.AluOpType.max,
                            )
                        if kc % 4 == 3:
                            j = kc // 4
                            nc.vector.tensor_tensor(
                                m2[:, j * SIMW : (j + 1) * SIMW],
                                m1[:, (2 * j) * SIMW : (2 * j + 1) * SIMW],
                                m1[:, (2 * j + 1) * SIMW : (2 * j + 2) * SIMW],
                                mybir.AluOpType.max,
                            )
                        if kc % 8 == 7:
                            j = kc // 8
                            nc.vector.tensor_tensor(
                                m3[:, j * SIMW : (j + 1) * SIMW],
                                m2[:, (2 * j) * SIMW : (2 * j + 1) * SIMW],
                                m2[:, (2 * j + 1) * SIMW : (2 * j + 2) * SIMW],
                                mybir.AluOpType.max,
                            )
                    rowmax = p1mx.tile([128, 1], f32, name="rowmax", tag="rowmax")
                    nc.vector.tensor_reduce(
                        rowmax[:], m3[:], mybir.AxisListType.X, mybir.AluOpType.max
                    )
                    onehot = p1oh.tile([128, K], f16, name="onehot", tag="onehot")
                    nc.vector.tensor_scalar(
                        onehot[:], simbuf[:], rowmax[:, 0:1], None, mybir.AluOpType.is_ge
                    )
                    nc.sync.dma_start(
                        onehot_dram[tt * 128 : (tt + 1) * 128, :], onehot[:]
                    )

            # ---------------- P2: segment sums via one-hot matmuls ----------------
            with (
                tc.tile_pool(name="p2oh", bufs=10) as p2oh,
                tc.tile_pool(name="p3sb", bufs=2) as p3sb,
                tc.tile_pool(name="p2st", bufs=4) as p2st,
                tc.tile_pool(name="p2ps", bufs=8, space="PSUM") as p2ps,
            ):
                for g in range(8):
                    segs = [
                        p2ps.tile([128, XW], f32, name=f"ps_seg{b}", tag="ps_seg")
                        for b in range(8)
                    ]
                    for tt in range(TT):
                        oh = p2oh.tile([128, 1024], f16, name="oh", tag="oh")
                        # Activation-hosted DMA queue: decouple from SP spills
                        nc.scalar.dma_start(
                            oh[:],
                            onehot_dram[
                                tt * 128 : (tt + 1) * 128, g * 1024 : (g + 1) * 1024
                            ],
                        )
                        for b in range(8):
                            nc.tensor.matmul(
                                segs[b][:],
                                oh[:, b * 128 : (b + 1) * 128],
                                xext[:, tt, :],
                                start=(tt == 0),
                                stop=(tt == TT - 1),
                            )
                    for b in range(8):
                        stg = p2st.tile([128, XW], f32, name="stg", tag="stg")
                        # alternate engines so the evac tail drains in parallel
                        if b % 2 == 0:
                            nc.scalar.copy(stg[:], segs[b][:])
                        else:
                            nc.vector.tensor_copy(stg[:], segs[b][:])
                        kt = g * 8 + b
                        nc.sync.dma_start(
                            partial_dram[kt * 128 : (kt + 1) * 128, :], stg[:]
                        )
                    # per-group ReduceScatter: overlaps later groups' matmuls.
                    # rank i receives rows [g*1024 + i*128, +128) -> ccout[g*128:(g+1)*128]
                    if globals().get("SKIP_COLLECTIVE", False):
                        nc.sync.dma_start(
                            ccout_dram[g * 128 : (g + 1) * 128, :],
                            partial_dram[g * 1024 : g * 1024 + 128, :],
                        )
                    else:
                        nc.gpsimd.collective_compute(
                            "ReduceScatter",
                            mybir.AluOpType.add,
                            replica_groups=[list(range(NCORES))],
                            ins=[partial_dram[g * 1024 : (g + 1) * 1024, :].opt()],
                            outs=[ccout_dram[g * 128 : (g + 1) * 128, :].opt()],
                        )
                    st = g
                    red = p3sb.tile([128, XW], f32, name="red", tag="red")
                    nc.sync.dma_start(red[:], ccout_dram[st * 128 : (st + 1) * 128, :])
                    dsum_h = p3sb.tile([128, C], f16, name="dsum_h", tag="dsum_h")
                    nc.sync.dma_start(dsum_h[:], dsum[st * 128 : (st + 1) * 128, :])
                    dnum_t = p3sb.tile([128, 1], f32, name="dnum_t", tag="dnum_t")
                    nc.sync.dma_start(dnum_t[:], dnum[st, :, :])
                    dsh_h = p3sb.tile([128, C], f16, name="dsh_h", tag="dsh_h")
                    nc.sync.dma_start(dsh_h[:], dsh[st * 128 : (st + 1) * 128, :])
                    dsum_t = p3sb.tile([128, C], f32, name="dsum_t", tag="dsum_t")
                    nc.vector.tensor_copy(dsum_t[:], dsum_h[:])
                    dsh_t = p3sb.tile([128, C], f32, name="dsh_t", tag="dsh_t")
                    nc.vector.tensor_copy(dsh_t[:], dsh_h[:])

                    cnt = red[:, C : C + 1]
                    maskb = p3sb.tile([128, 1], f32, name="maskb", tag="maskb")
                    nc.vector.tensor_scalar(
                        maskb[:], cnt, 0.0, None, mybir.AluOpType.is_gt
                    )
                    mask001 = p3sb.tile([128, 1], f32, name="mask001", tag="mask001")
                    nc.vector.tensor_scalar(
                        mask001[:], cnt, 0.0, 1.0 - MOM,
                        mybir.AluOpType.is_gt, mybir.AluOpType.mult,
                    )
                    tmp = p3sb.tile([128, C], f32, name="tmp", tag="tmp")
                    nc.vector.tensor_tensor(
                        tmp[:], red[:, 0:C], dsum_t[:], mybir.AluOpType.subtract
                    )
                    nc.vector.tensor_scalar(
                        tmp[:], tmp[:], mask001[:, 0:1], None, mybir.AluOpType.mult
                    )
                    nsum = p3sb.tile([128, C], f32, name="nsum", tag="nsum")
                    nc.vector.tensor_tensor(
                        nsum[:], tmp[:], dsum_t[:], mybir.AluOpType.add
                    )
                    n0 = p3sb.tile([128, 1], f32, name="n0", tag="n0")
                    nc.vector.tensor_tensor(
                        n0[:], cnt, dnum_t[:], mybir.AluOpType.subtract
                    )
                    nc.vector.tensor_tensor(
                        n0[:], n0[:], mask001[:], mybir.AluOpType.mult
                    )
                    nnum = p3sb.tile([128, 1], f32, name="nnum", tag="nnum")
                    nc.vector.tensor_tensor(
                        nnum[:], n0[:], dnum_t[:], mybir.AluOpType.add
                    )
                    rec = p3sb.tile([128, 1], f32, name="recq", tag="recq")
                    nc.vector.reciprocal(rec[:], nnum[:])
                    q = p3sb.tile([128, C], f32, name="q", tag="q")
                    nc.vector.tensor_scalar(
                        q[:], nsum[:], rec[:, 0:1], None, mybir.AluOpType.mult
                    )
                    nc.vector.tensor_tensor(
                        q[:], q[:], dsh_t[:], mybir.AluOpType.subtract
                    )
                    nc.vector.tensor_scalar(
                        q[:], q[:], maskb[:, 0:1], None, mybir.AluOpType.mult
                    )
                    outt = p3sb.tile([128, C], f32, name="outt", tag="outt")
                    nc.vector.tensor_tensor(
                        outt[:], q[:], dsh_t[:], mybir.AluOpType.add
                    )
                    nc.sync.dma_start(
                        out_shard[st * 128 : (st + 1) * 128, :], outt[:]
                    )

    nc.compile()
    return nc


def _shard_rows(i):
    """Global dictionary rows owned by core i: the i-th 128-block of each group."""
    return [(g * KSH + i * 128, g * KSH + i * 128 + 128) for g in range(KSH // 128)]


def shard_inputs(feature, dictionary, dictionary_sum, dictionary_num):
    feature16 = feature.astype(np.float16)
    dictionary16 = dictionary.astype(np.float16)
    dsum16 = dictionary_sum.astype(np.float16)
    in_maps = []
    for i in range(NCORES):
        rows = _shard_rows(i)
        dsum_i = np.concatenate([dsum16[a:b] for a, b in rows], axis=0)
        dsh_i = np.concatenate([dictionary16[a:b] for a, b in rows], axis=0)
        dnum_i = np.concatenate([dictionary_num[a:b] for a, b in rows], axis=0)
        in_maps.append(
            {
                "feat": np.ascontiguousarray(feature16[i * NSH : (i + 1) * NSH]),
                "dic": dictionary16,
                "dsum": np.ascontiguousarray(dsum_i),
                "dnum": np.ascontiguousarray(dnum_i).reshape(KSH // 128, 128, 1),
                "dsh": np.ascontiguousarray(dsh_i),
            }
        )
    return in_maps


def unshard_output(results):
    out = np.empty((K, C), np.float32)
    for i in range(NCORES):
        rows = _shard_rows(i)
        for g, (a, b) in enumerate(rows):
            out[a:b] = results[i]["out_shard"][g * 128 : (g + 1) * 128]
    return out


def kernel(feature, dictionary, dictionary_sum, dictionary_num):
    from concourse import bass_utils

    feature = np.ascontiguousarray(feature, dtype=np.float32)
    dictionary = np.ascontiguousarray(dictionary, dtype=np.float32)
    dictionary_sum = np.ascontiguousarray(dictionary_sum, dtype=np.float32)
    dictionary_num = np.ascontiguousarray(dictionary_num, dtype=np.float32)

    nc = _build()
    in_maps = shard_inputs(feature, dictionary, dictionary_sum, dictionary_num)
    res = bass_utils.run_bass_kernel_spmd(nc, in_maps, core_ids=list(range(NCORES)))
    return unshard_output(res.results).astype(np.float32)


# revision 13
# speedup vs baseline: 1.2207x; 1.2207x over previous
"""VQ-EMA codebook update kernel for Trainium2, 8 NeuronCores.

Strategy (data-parallel over tokens, standard VQ-EMA sync):
  - Each core: N/8 = 4096 tokens; [K=8192, C=384] dictionary replicated.
  - Key insight: argmax_k <x/|x|, d_k/|d_k|> == argmax_k <x, d_k/|d_k|>, so
    features are never normalized; only the dictionary is (P0).
  - All sim-side data in fp16: ndT [C,K] fp16 (48KB/part), xT fp16 chunks.
  - P1 per 128-token tile: cast x->fp16 into resident xext (x | ones col),
    transpose x chunks on PE, sim = xT.T @ ndT in fp16 (1 cyc/row), PSUM->
    SBUF fp16 evacuations split ScalarE/DVE, rowmax via 2x tensor_tensor max
    tree + small 1x reduce, onehot = (sim >= rowmax) fp16 at 4x DVE rate,
    spilled to DRAM (64MB; xext stays resident).
  - P2: segment sums: per K-group, accumulate onehot.T @ xext over all token
    tiles in PSUM -> partial [K, C+1] (feature sums + counts).
  - ReduceScatter(add) per group across 8 cores -> each core owns [1024,C+1]
    shard rows; EMA update + where(used) blend on-core (P3).
Host: shards inputs, concatenates the 8 output shards.
"""

import sys

sys.path.insert(0, "/opt/trn_rl_repo")

import functools

import numpy as np

N = 32768
C = 384
K = 8192
NCORES = 8
NSH = N // NCORES  # 4096 tokens per core
KSH = K // NCORES  # 1024 dict rows per core
TT = NSH // 128  # 32 token tiles per core
KT = K // 128  # 64 K tiles
CB = C // 128  # 3 contraction chunks
SIMW = 512  # sim matmul free width (PSUM bank, fp32)
NSIMW = K // SIMW  # 16 chunks
XW = C + 1  # 385: x plus ones column
MOM = 0.99
N_SE_EVAC = 12  # sim chunks evacuated on ScalarE; rest on DVE


@functools.cache
def _build():
    import concourse.bacc as bacc
    import concourse.masks as masks
    import concourse.mybir as mybir
    import concourse.tile as tile

    f32 = mybir.dt.float32
    f16 = mybir.dt.float16

    nc = bacc.Bacc("TRN2", target_bir_lowering=False, debug=False, num_devices=NCORES)

    # fp16 inputs: the per-execution input staging costs ~17ns/byte, and the
    # kernel quantizes feat/dic to fp16 internally anyway. dsum/dsh at fp16
    # add ~2e-4 relative error, well within tolerance.
    feat = nc.dram_tensor("feat", [NSH, C], f16, kind="ExternalInput").ap()
    dic = nc.dram_tensor("dic", [K, C], f16, kind="ExternalInput").ap()
    dsum = nc.dram_tensor("dsum", [KSH, C], f16, kind="ExternalInput").ap()
    dnum = nc.dram_tensor("dnum", [KSH // 128, 128, 1], f32, kind="ExternalInput").ap()
    dsh = nc.dram_tensor("dsh", [KSH, C], f16, kind="ExternalInput").ap()
    out_shard = nc.dram_tensor("out_shard", [KSH, C], f32, kind="ExternalOutput").ap()

    with tile.TileContext(nc) as tc:
        with (
            tc.tile_pool(name="constp", bufs=1) as constp,
            tc.tile_pool(name="mainp", bufs=1) as mainp,
            tc.tile_pool(name="dramp", bufs=1, space="DRAM") as dramp,
        ):
            ident = constp.tile([128, 128], f16, name="ident")
            masks.make_identity(nc, ident[:])

            # Persistent SBUF tensors
            ndT = [
                mainp.tile([128, K], f16, name=f"ndT{c}", uniquify=False)
                for c in range(CB)
            ]
            # xext: fp16 features + ones column, resident across P1/P2
            xext = mainp.tile([128, TT, XW], f16, name="xext", uniquify=False)
            nc.vector.memset(xext[:, :, C:XW], 1.0)

            # DRAM scratch
            onehot_dram = dramp.tile([NSH, K], f16, name="onehot_dram")
            partial_dram = dramp.tile([K, XW], f32, name="partial_dram")
            ccout_dram = dramp.tile([KSH, XW], f32, name="ccout_dram")

            # ---------------- P0: dictionary normalize + transpose ----------------
            with (
                tc.tile_pool(name="p0sb", bufs=3) as p0sb,
                tc.tile_pool(name="p0sc", bufs=2) as p0sc,
                tc.tile_pool(name="p0ps", bufs=2, space="PSUM") as p0ps,
            ):
                for dt_i in range(KT):
                    d = p0sb.tile([128, C], f16, name="d", tag="d")
                    nc.sync.dma_start(d[:], dic[dt_i * 128 : (dt_i + 1) * 128, :])
                    sq = p0sc.tile([128, C], f32, name="sq", tag="sq")
                    ss = p0sc.tile([128, 1], f32, name="ss", tag="ss")
                    nc.scalar.activation(
                        sq[:], d[:], mybir.ActivationFunctionType.Square, accum_out=ss[:]
                    )
                    # r ~= 1/sqrt(ss): reciprocal+sqrt then one Newton step
                    rec = p0sc.tile([128, 1], f32, name="rec", tag="rec")
                    r0 = p0sc.tile([128, 1], f32, name="r0", tag="r0")
                    t = p0sc.tile([128, 1], f32, name="t", tag="t")
                    r = p0sc.tile([128, 1], f32, name="r", tag="r")
                    nc.vector.reciprocal(rec[:], ss[:])
                    nc.scalar.sqrt(r0[:], rec[:])
                    nc.vector.tensor_tensor(t[:], r0[:], r0[:], mybir.AluOpType.mult)
                    nc.vector.tensor_tensor(t[:], t[:], ss[:], mybir.AluOpType.mult)
                    nc.vector.tensor_scalar(
                        t[:], t[:], -0.5, 1.5, mybir.AluOpType.mult, mybir.AluOpType.add
                    )
                    nc.vector.tensor_tensor(r[:], r0[:], t[:], mybir.AluOpType.mult)
                    nd = p0sb.tile([128, C], f16, name="nd", tag="nd")
                    # normalize on DVE (2x single-src mode) to keep ScalarE light
                    nc.vector.tensor_scalar(
                        nd[:], d[:], r[:, 0:1], None, mybir.AluOpType.mult
                    )
                    for c in range(CB):
                        pst = p0ps.tile([128, 128], f16, name="pst", tag="pst")
                        nc.tensor.transpose(pst[:], nd[:, c * 128 : (c + 1) * 128], ident[:])
                        nc.vector.tensor_copy(
                            ndT[c][:, dt_i * 128 : (dt_i + 1) * 128], pst[:]
                        )

            # ---------------- P1: sim + one-hot per token tile ----------------
            with (
                tc.tile_pool(name="p1sb", bufs=4) as p1sb,
                tc.tile_pool(name="p1xt", bufs=3) as p1xt,
                tc.tile_pool(name="p1sim", bufs=2) as p1sim,
                tc.tile_pool(name="p1mx", bufs=2) as p1mx,
                tc.tile_pool(name="p1oh", bufs=2) as p1oh,
                tc.tile_pool(name="p1ps", bufs=4, space="PSUM") as p1ps,
                tc.tile_pool(name="p1pst", bufs=3, space="PSUM") as p1pst,
            ):
                for tt in range(TT):
                    # dense fp16 load, then a cheap 4x-mode copy into the
                    # strided resident xext slot (strided DMA writes are slow)
                    x = p1sb.tile([128, C], f16, name="x", tag="x")
                    nc.sync.dma_start(x[:], feat[tt * 128 : (tt + 1) * 128, :])
                    # xext copy is only needed by P2; off the critical path
                    nc.vector.tensor_copy(xext[:, tt, 0:C], x[:])

                    xT = []
                    for c in range(CB):
                        pst = p1pst.tile([128, 128], f16, name="pstx", tag="pstx")
                        nc.tensor.transpose(
                            pst[:], x[:, c * 128 : (c + 1) * 128], ident[:]
                        )
                        xc = p1xt.tile([128, 128], f16, name="xc", tag=f"xc{c}")
                        nc.scalar.copy(xc[:], pst[:])
                        xT.append(xc)

                    simbuf = p1sim.tile([128, K], f16, name="simbuf", tag="simbuf")
                    m1 = p1mx.tile([128, K // 2], f16, name="m1", tag="m1")
                    m2 = p1mx.tile([128, K // 4], f16, name="m2", tag="m2")
                    m3 = p1mx.tile([128, K // 8], f16, name="m3", tag="m3")
                    for kc in range(NSIMW):
                        ps = p1ps.tile([128, SIMW], f32, name="ps_sim", tag="ps_sim")
                        for c in range(CB):
                            nc.tensor.matmul(
                                ps[:],
                                xT[c][:],
                                ndT[c][:, kc * SIMW : (kc + 1) * SIMW],
                                start=(c == 0),
                                stop=(c == CB - 1),
                            )
                        dst = simbuf[:, kc * SIMW : (kc + 1) * SIMW]
                        if kc < N_SE_EVAC:
                            nc.scalar.copy(dst, ps[:])
                        else:
                            nc.vector.tensor_copy(dst, ps[:])
                        # pairwise max as soon as a chunk pair lands: small 2x
                        # ops pipeline with the matmuls and avoid DVE HOL stalls
                        if kc % 2 == 1:
                            j = kc // 2
                            nc.vector.tensor_tensor(
                                m1[:, j * SIMW : (j + 1) * SIMW],
                                simbuf[:, (kc - 1) * SIMW : kc * SIMW],
                                dst,
                                mybir.AluOpType.max,
                            )
                        if kc % 4 == 3:
                            j = kc // 4
                            nc.vector.tensor_tensor(
                                m2[:, j * SIMW : (j + 1) * SIMW],
                                m1[:, (2 * j) * SIMW : (2 * j + 1) * SIMW],
                                m1[:, (2 * j + 1) * SIMW : (2 * j + 2) * SIMW],
                                mybir.AluOpType.max,
                            )
                        if kc % 8 == 7:
                            j = kc // 8
                            nc.vector.tensor_tensor(
                                m3[:, j * SIMW : (j + 1) * SIMW],
                                m2[:, (2 * j) * SIMW : (2 * j + 1) * SIMW],
                                m2[:, (2 * j + 1) * SIMW : (2 * j + 2) * SIMW],
                                mybir.AluOpType.max,
                            )
                    rowmax = p1mx.tile([128, 1], f32, name="rowmax", tag="rowmax")
                    nc.vector.tensor_reduce(
                        rowmax[:], m3[:], mybir.AxisListType.X, mybir.AluOpType.max
                    )
                    onehot = p1oh.tile([128, K], f16, name="onehot", tag="onehot")
                    nc.vector.tensor_scalar(
                        onehot[:], simbuf[:], rowmax[:, 0:1], None, mybir.AluOpType.is_ge
                    )
                    nc.sync.dma_start(
                        onehot_dram[tt * 128 : (tt + 1) * 128, :], onehot[:]
                    )

            # ---------------- P2: segment sums via one-hot matmuls ----------------
            with (
                tc.tile_pool(name="p2oh", bufs=10) as p2oh,
                tc.tile_pool(name="p3sb", bufs=2) as p3sb,
                tc.tile_pool(name="p2st", bufs=4) as p2st,
                tc.tile_pool(name="p2ps", bufs=8, space="PSUM") as p2ps,
            ):
                for g in range(8):
                    segs = [
                        p2ps.tile([128, XW], f32, name=f"ps_seg{b}", tag="ps_seg")
                        for b in range(8)
                    ]
                    for tt in range(TT):
                        oh = p2oh.tile([128, 1024], f16, name="oh", tag="oh")
                        # Activation-hosted DMA queue: decouple from SP spills
                        nc.scalar.dma_start(
                            oh[:],
                            onehot_dram[
                                tt * 128 : (tt + 1) * 128, g * 1024 : (g + 1) * 1024
                            ],
                        )
                        for b in range(8):
                            nc.tensor.matmul(
                                segs[b][:],
                                oh[:, b * 128 : (b + 1) * 128],
                                xext[:, tt, :],
                                start=(tt == 0),
                                stop=(tt == TT - 1),
                            )
                    for b in range(8):
                        stg = p2st.tile([128, XW], f32, name="stg", tag="stg")
                        # alternate engines so the evac tail drains in parallel
                        if b % 2 == 0:
                            nc.scalar.copy(stg[:], segs[b][:])
                        else:
                            nc.vector.tensor_copy(stg[:], segs[b][:])
                        kt = g * 8 + b
                        nc.sync.dma_start(
                            partial_dram[kt * 128 : (kt + 1) * 128, :], stg[:]
                        )
                    # per-group ReduceScatter: overlaps later groups' matmuls.
                    # rank i receives rows [g*1024 + i*128, +128) -> ccout[g*128:(g+1)*128]
                    if globals().get("SKIP_COLLECTIVE", False):
                        nc.sync.dma_start(
                            ccout_dram[g * 128 : (g + 1) * 128, :],
                            partial_dram[g * 1024 : g * 1024 + 128, :],
                        )
                    else:
                        nc.gpsimd.collective_compute(
                            "ReduceScatter",
                            mybir.AluOpType.add,
                            replica_groups=[list(range(NCORES))],
                            ins=[partial_dram[g * 1024 : (g + 1) * 1024, :].opt()],
                            outs=[ccout_dram[g * 128 : (g + 1) * 128, :].opt()],
                        )
                    st = g
                    red = p3sb.tile([128, XW], f32, name="red", tag="red")
                    nc.sync.dma_start(red[:], ccout_dram[st * 128 : (st + 1) * 128, :])
                    dsum_h = p3sb.tile([128, C], f16, name="dsum_h", tag="dsum_h")
                    nc.sync.dma_start(dsum_h[:], dsum[st * 128 : (st + 1) * 128, :])
                    dnum_t = p3sb.tile([128, 1], f32, name="dnum_t", tag="dnum_t")
                    nc.sync.dma_start(dnum_t[:], dnum[st, :, :])
                    dsh_h = p3sb.tile([128, C], f16, name="dsh_h", tag="dsh_h")
                    nc.sync.dma_start(dsh_h[:], dsh[st * 128 : (st + 1) * 128, :])
                    dsum_t = p3sb.tile([128, C], f32, name="dsum_t", tag="dsum_t")
                    nc.vector.tensor_copy(dsum_t[:], dsum_h[:])
                    dsh_t = p3sb.tile([128, C], f32, name="dsh_t", tag="dsh_t")
                    nc.vector.tensor_copy(dsh_t[:], dsh_h[:])

                    cnt = red[:, C : C + 1]
                    maskb = p3sb.tile([128, 1], f32, name="maskb", tag="maskb")
                    nc.vector.tensor_scalar(
                        maskb[:], cnt, 0.0, None, mybir.AluOpType.is_gt
                    )
                    mask001 = p3sb.tile([128, 1], f32, name="mask001", tag="mask001")
                    nc.vector.tensor_scalar(
                        mask001[:], cnt, 0.0, 1.0 - MOM,
                        mybir.AluOpType.is_gt, mybir.AluOpType.mult,
                    )
                    tmp = p3sb.tile([128, C], f32, name="tmp", tag="tmp")
                    nc.vector.tensor_tensor(
                        tmp[:], red[:, 0:C], dsum_t[:], mybir.AluOpType.subtract
                    )
                    nc.vector.tensor_scalar(
                        tmp[:], tmp[:], mask001[:, 0:1], None, mybir.AluOpType.mult
                    )
                    nsum = p3sb.tile([128, C], f32, name="nsum", tag="nsum")
                    nc.vector.tensor_tensor(
                        nsum[:], tmp[:], dsum_t[:], mybir.AluOpType.add
                    )
                    n0 = p3sb.tile([128, 1], f32, name="n0", tag="n0")
                    nc.vector.tensor_tensor(
                        n0[:], cnt, dnum_t[:], mybir.AluOpType.subtract
                    )
                    nc.vector.tensor_tensor(
                        n0[:], n0[:], mask001[:], mybir.AluOpType.mult
                    )
                    nnum = p3sb.tile([128, 1], f32, name="nnum", tag="nnum")
                    nc.vector.tensor_tensor(
                        nnum[:], n0[:], dnum_t[:], mybir.AluOpType.add
                    )
                    rec = p3sb.tile([128, 1], f32, name="recq", tag="recq")
                    nc.vector.reciprocal(rec[:], nnum[:])
                    q = p3sb.tile([128, C], f32, name="q", tag="q")
                    nc.vector.tensor_scalar(
                        q[:], nsum[:], rec[:, 0:1], None, mybir.AluOpType.mult
                    )
                    nc.vector.tensor_tensor(
                        q[:], q[:], dsh_t[:], mybir.AluOpType.subtract
                    )
                    nc.vector.tensor_scalar(
                        q[:], q[:], maskb[:, 0:1], None, mybir.AluOpType.mult
                    )
                    outt = p3sb.tile([128, C], f32, name="outt", tag="outt")
                    nc.vector.tensor_tensor(
                        outt[:], q[:], dsh_t[:], mybir.AluOpType.add
                    )
                    nc.sync.dma_start(
                        out_shard[st * 128 : (st + 1) * 128, :], outt[:]
                    )

    nc.compile()
    return nc


def _shard_rows(i):
    """Global dictionary rows owned by core i: the i-th 128-block of each group."""
    return [(g * KSH + i * 128, g * KSH + i * 128 + 128) for g in range(KSH // 128)]


def shard_inputs(feature, dictionary, dictionary_sum, dictionary_num):
    feature16 = feature.astype(np.float16)
    dictionary16 = dictionary.astype(np.float16)
    dsum16 = dictionary_sum.astype(np.float16)
    in_maps = []
    for i in range(NCORES):
        rows = _shard_rows(i)
        dsum_i = np.concatenate([dsum16[a:b] for a, b in rows], axis=0)
        dsh_i = np.concatenate([dictionary16[a:b] for a, b in rows], axis=0)
        dnum_i = np.concatenate([dictionary_num[a:b] for a, b in rows], axis=0)
        in_maps.append(
            {
                "feat": np.ascontiguousarray(feature16[i * NSH : (i + 1) * NSH]),
                "dic": dictionary16,
                "dsum": np.ascontiguousarray(dsum_i),
                "dnum": np.ascontiguousarray(dnum_i).reshape(KSH // 128, 128, 1),
                "dsh": np.ascontiguousarray(dsh_i),
            }
        )
    return in_maps


def unshard_output(results):
    out = np.empty((K, C), np.float32)
    for i in range(NCORES):
        rows = _shard_rows(i)
        for g, (a, b) in enumerate(rows):
            out[a:b] = results[i]["out_shard"][g * 128 : (g + 1) * 128]
    return out


def kernel(feature, dictionary, dictionary_sum, dictionary_num):
    from concourse import bass_utils

    feature = np.ascontiguousarray(feature, dtype=np.float32)
    dictionary = np.ascontiguousarray(dictionary, dtype=np.float32)
    dictionary_sum = np.ascontiguousarray(dictionary_sum, dtype=np.float32)
    dictionary_num = np.ascontiguousarray(dictionary_num, dtype=np.float32)

    nc = _build()
    in_maps = shard_inputs(feature, dictionary, dictionary_sum, dictionary_num)
    res = bass_utils.run_bass_kernel_spmd(nc, in_maps, core_ids=list(range(NCORES)))
    return unshard_output(res.results).astype(np.float32)
